# revision 1
# baseline (speedup 1.0000x reference)
"""Trainium2 Bass kernel for nn_EngramShortConv (RMSNorm + depthwise dilated
causal conv1d + silu), 8-core SPMD.

  x: [B=4, L=4096, HC=4, D=1024] fp32 -> y same shape/dtype.

Sharding: 16 independent (b, hc) groups, 2 per NeuronCore, zero communication.

Per core, per 512-token chunk (natural [token, d] layout from HBM, fp16
on-chip, fp32 PSUM accumulation):
  1. stats: square ops with 1/D folded in accumulate to ms per token
     (split DVE STT / ACT Square, same ACT table as Silu); r =
     rsqrt(ms+eps) via bit-trick + 1 Newton step, DVE only -- no ACT
     table swaps anywhere in the kernel (rsqrt rel err ~2e-3, well
     inside the 2e-2 budget).
  2. pass1 (PE): Z[d, t] = X_blk^T @ diag(r) per 128x128 block -- the
     transpose to channel-major with the RMSNorm scale folded in for free.
     DVE/ACT copy PSUM -> SBUF fp16 with a 6-column halo from the previous
     chunk (single merged copy).
  3. pass2 (PE): the depthwise conv as 4 PSUM-accumulated matmuls
     diag(conv_w[k] * norm_w) @ Z[:, t - 6 + 2k], with the norm affine
     weight folded into the diagonals on the host.
  4. ACT Silu reads conv PSUM -> fp16; pass3 (PE) transposes back via
     identity; DVE/ACT copy to SBUF; DMA out.

I/O precision: host casts x to fp16 before upload (the device would round
to fp16 anyway; halves input DMA) and the device returns fp16 y upcast to
fp32 on host. End-to-end scale-relative error ~1e-3.
"""

import sys

if "/opt/trn_rl_repo" not in sys.path:
    sys.path.insert(0, "/opt/trn_rl_repo")

import numpy as np

B, L, HC, D = 4, 4096, 4, 1024
K, DIL = 4, 2
EPS = 1e-5
PAD = (K - 1) * DIL  # 6
NCORES = 8
NGROUPS = B * HC     # 16
GPC = NGROUPS // NCORES  # 2 groups per core

# tunables
TCH = 512            # tokens per chunk (= matmul moving free dim)
CPAIR = 2            # chunks whose stats are batched
SQ_ENGINES = ("vector", "vector", "vector", "act")  # per square block
OUTCOPY_ACT = 3      # of 4 blks per chunk, how many outcopies go to ACT
ZCOPY_ACT = 0        # of 8 dsubs per chunk, how many zcopies go to ACT

_prog_cache = {}


def build_program(L_=L, gpc=GPC, tch=TCH, cpair=CPAIR,
                  sq_engines=SQ_ENGINES, outcopy_act=OUTCOPY_ACT,
                  zcopy_act=ZCOPY_ACT):
    """Build the per-core Bacc program. Same program on all cores (SPMD)."""
    import concourse.bacc as bacc
    import concourse.tile as tile
    from concourse import mybir

    f32 = mybir.dt.float32
    f16 = mybir.dt.float16
    i32 = mybir.dt.int32
    AF = mybir.ActivationFunctionType
    ALU = mybir.AluOpType

    nblk = tch // 128
    dsub = D // 128
    nchunks = L_ // tch
    assert tch % 128 == 0 and L_ % tch == 0 and D % 128 == 0

    nc = bacc.Bacc()
    xin = nc.declare_dram_parameter("xin", [gpc, L_, D], f16, isOutput=False)
    wdg = nc.declare_dram_parameter("wdg", [gpc, K, dsub, 128, 128], f16,
                                    isOutput=False)
    idn = nc.declare_dram_parameter("idn", [128, 128], f16, isOutput=False)
    yout = nc.declare_dram_parameter("yout", [gpc, L_, D], f16, isOutput=True)

    # views: token index t = c*tch + blk*128 + p
    xv = xin[:].rearrange("g (c blk p) d -> g c p blk d", blk=nblk, p=128)
    yv = yout[:].rearrange("g (c blk p) d -> g c p blk d", blk=nblk, p=128)
    wv = wdg[:].rearrange("g k s p m -> p g k s m")

    with tile.TileContext(nc) as tc:
        with (
            tc.tile_pool(name="pconst", bufs=1) as pconst,
            tc.tile_pool(name="px", bufs=8) as px,
            tc.tile_pool(name="pstat", bufs=3) as pstat,
            tc.tile_pool(name="pz", bufs=4) as pz,
            tc.tile_pool(name="py", bufs=3) as py,
            tc.tile_pool(name="po", bufs=3) as po,
            tc.tile_pool(name="pp1", bufs=2, space="PSUM") as pp1,
            tc.tile_pool(name="pp2", bufs=2, space="PSUM") as pp2,
            tc.tile_pool(name="pp3", bufs=2, space="PSUM") as pp3,
        ):
            ident = pconst.tile([128, 128], f16)
            nc.sync.dma_start(out=ident[:], in_=idn[:])
            wsb = pconst.tile([128, gpc, K, dsub, 128], f16)
            nc.sync.dma_start(out=wsb[:], in_=wv)

            zt_prev = None
            for g in range(gpc):
                for c0 in range(0, nchunks, cpair):
                    cs = list(range(c0, min(c0 + cpair, nchunks)))
                    ncs = len(cs)
                    # ---- load chunks (natural [token, d] layout) ----
                    xhs = []
                    for c in cs:
                        xh = px.tile([128, nblk, D], f16, tag="xh")
                        nc.sync.dma_start(out=xh[:], in_=xv[g, c])
                        xhs.append(xh)

                    # ---- stats: ms = mean(x^2), batched per pair ----
                    ssq = pstat.tile([128, ncs, nblk], f32, tag="ssq")
                    for j in range(ncs):
                        for blk in range(nblk):
                            eng = sq_engines[blk % len(sq_engines)]
                            scr = pstat.tile([128, D], f16, tag="scr")
                            if eng == "act":
                                nc.scalar.activation(
                                    out=scr[:], in_=xhs[j][:, blk, :],
                                    func=AF.Square, scale=float(D) ** -0.5,
                                    accum_out=ssq[:, j, blk:blk + 1])
                            else:
                                nc.vector.scalar_tensor_tensor(
                                    out=scr[:], in0=xhs[j][:, blk, :],
                                    scalar=1.0 / D, in1=xhs[j][:, blk, :],
                                    op0=ALU.mult, op1=ALU.mult,
                                    accum_out=ssq[:, j, blk:blk + 1])
                    # r = rsqrt(ms+eps): bit trick + 1 Newton step on
                    # DVE (keeps Sqrt out of ACT -> zero table swaps)
                    v = pstat.tile([128, ncs, nblk], f32, tag="v")
                    nc.vector.tensor_scalar(
                        out=v[:], in0=ssq[:], scalar1=EPS, scalar2=None,
                        op0=ALU.add)
                    r = pstat.tile([128, ncs, nblk], f32, tag="r")
                    nc.vector.tensor_scalar(
                        out=r[:].bitcast(i32), in0=v[:].bitcast(i32),
                        scalar1=1, scalar2=None, op0=ALU.arith_shift_right)
                    nc.vector.tensor_scalar(
                        out=r[:].bitcast(i32), in0=r[:].bitcast(i32),
                        scalar1=-1, scalar2=0x5F3759DF,
                        op0=ALU.mult, op1=ALU.add)
                    for _ in range(1):
                        yy = pstat.tile([128, ncs, nblk], f32, tag="yy")
                        nc.vector.tensor_tensor(
                            out=yy[:], in0=r[:], in1=r[:], op=ALU.mult)
                        nc.vector.tensor_tensor(
                            out=yy[:], in0=yy[:], in1=v[:], op=ALU.mult)
                        nc.vector.tensor_scalar(
                            out=yy[:], in0=yy[:], scalar1=-0.5, scalar2=1.5,
                            op0=ALU.mult, op1=ALU.add)
                        rn = pstat.tile([128, ncs, nblk], f32, tag="rn")
                        nc.vector.tensor_tensor(
                            out=rn[:], in0=r[:], in1=yy[:], op=ALU.mult)
                        r = rn

                    # ---- pass1 per chunk: Z[d, t] = X^T diag(r) ----
                    zts = []
                    for j, c in enumerate(cs):
                        xh = xhs[j]
                        drt = pstat.tile([128, nblk, 128], f16, tag="drt")
                        for blk in range(nblk):
                            nc.vector.tensor_scalar_mul(
                                out=drt[:, blk, :], in0=ident[:],
                                scalar1=r[:, j, blk:blk + 1])

                        zt = pz.tile([128, dsub, PAD + tch], f16, tag="zt")
                        if c == 0:
                            nc.vector.memset(zt[:, :, 0:PAD], 0.0)
                        else:
                            nc.vector.tensor_copy(
                                out=zt[:, :, 0:PAD],
                                in_=zt_prev[:, :, tch:tch + PAD])
                        for s in range(dsub):
                            zp = pp1.tile([128, tch], f32, tag="zp")
                            for blk in range(nblk):
                                nc.tensor.matmul(
                                    zp[:, blk * 128:(blk + 1) * 128],
                                    lhsT=xh[:, blk, s * 128:(s + 1) * 128],
                                    rhs=drt[:, blk, :],
                                    start=True, stop=True)
                            if s < zcopy_act:
                                nc.scalar.copy(
                                    out=zt[:, s, PAD:PAD + tch], in_=zp[:])
                            else:
                                nc.vector.tensor_copy(
                                    out=zt[:, s, PAD:PAD + tch], in_=zp[:])
                        zt_prev = zt
                        zts.append(zt)

                    # ---- pass2 paired: conv matmuls share ldweights ----
                    yhs = [py.tile([128, dsub, tch], f16, tag=f"yh{j}",
                                   name=f"yh{j}_{g}_{c0}")
                           for j in range(ncs)]
                    for s in range(dsub):
                        yps = [pp2.tile([128, tch], f32, tag=f"yp{j}",
                                        name=f"yp{j}_{g}_{c0}_{s}")
                               for j in range(ncs)]
                        for k in range(K):
                            for j in range(ncs):
                                nc.tensor.matmul(
                                    yps[j][:],
                                    lhsT=wsb[:, g, k, s, :],
                                    rhs=zts[j][:, s, k * DIL:k * DIL + tch],
                                    start=(k == 0), stop=(k == K - 1))
                        for j in range(ncs):
                            nc.scalar.activation(out=yhs[j][:, s, :],
                                                 in_=yps[j][:], func=AF.Silu)

                    # ---- pass3 per chunk: transpose back + copy + store ----
                    for j, c in enumerate(cs):
                        yh = yhs[j]
                        yo = po.tile([128, nblk, D], f16, tag="yo")
                        for blk in range(nblk):
                            on_act = blk < outcopy_act
                            for half in range(2):
                                tp = pp3.tile([128, D // 2], f32, tag="tp")
                                for sh in range(dsub // 2):
                                    s = half * (dsub // 2) + sh
                                    nc.tensor.matmul(
                                        tp[:, sh * 128:(sh + 1) * 128],
                                        lhsT=yh[:, s,
                                                blk * 128:(blk + 1) * 128],
                                        rhs=ident[:],
                                        start=True, stop=True)
                                dst = yo[:, blk,
                                         half * (D // 2):(half + 1) * (D // 2)]
                                if on_act:
                                    nc.scalar.copy(out=dst, in_=tp[:])
                                else:
                                    nc.vector.tensor_copy(out=dst, in_=tp[:])
                        nc.sync.dma_start(out=yv[g, c], in_=yo[:])
    nc.compile()
    return nc


def _host_pack(x, norm_weight, conv_weight):
    """Shard inputs across cores; fold norm weight into conv diagonals."""
    dsub = D // 128
    xg = np.ascontiguousarray(x.transpose(0, 2, 1, 3)).reshape(NGROUPS, L, D)
    xg = xg.astype(np.float16)
    conv_w = conv_weight.reshape(HC, D, K)            # [hc, d, k]
    weff = conv_w * norm_weight[:, :, None]           # [hc, d, k]
    wr = weff.transpose(0, 2, 1).reshape(HC, K, dsub, 128)  # [hc, k, s, i]
    eye = np.eye(128, dtype=np.float32)
    wdiag = (wr[..., None] * eye).astype(np.float16)  # [hc, K, s, 128, 128]
    idn = eye.astype(np.float16)

    in_maps = []
    for i in range(NCORES):
        gs = [i * GPC + j for j in range(GPC)]
        in_maps.append({
            "xin": np.ascontiguousarray(xg[gs[0]:gs[-1] + 1]),
            "wdg": np.ascontiguousarray(
                np.stack([wdiag[g % HC] for g in gs])),
            "idn": idn,
        })
    return in_maps


def _host_unpack(results):
    ys = np.concatenate([r["yout"] for r in results], axis=0)  # [16, L, D]
    y = ys.reshape(B, HC, L, D).transpose(0, 2, 1, 3)
    return np.ascontiguousarray(y.astype(np.float32))


def _get_prog():
    key = (L, GPC, TCH, CPAIR, SQ_ENGINES, OUTCOPY_ACT, ZCOPY_ACT)
    if key not in _prog_cache:
        _prog_cache[key] = build_program()
    return _prog_cache[key]


def kernel(x, norm_weight, conv_weight, _trace=False, _trace_kwargs=None):
    from concourse.bass_utils import run_bass_kernel_spmd

    x = np.asarray(x, dtype=np.float32)
    norm_weight = np.asarray(norm_weight, dtype=np.float32)
    conv_weight = np.asarray(conv_weight, dtype=np.float32)

    nc = _get_prog()
    in_maps = _host_pack(x, norm_weight, conv_weight)
    res = run_bass_kernel_spmd(
        nc, in_maps, list(range(NCORES)),
        trace=_trace, **(_trace_kwargs or {}))
    out = _host_unpack(res.results)
    if _trace:
        return out, res
    return out



# revision 4
# speedup vs baseline: 1.1651x; 1.1651x over previous
"""Trainium2 Bass kernel for nn_EngramShortConv (RMSNorm + depthwise dilated
causal conv1d + silu), 8-core SPMD.

  x: [B=4, L=4096, HC=4, D=1024] fp32 -> y same shape/dtype.

Sharding: 16 independent (b, hc) groups, 2 per NeuronCore, zero communication.

v2: output is written CHANNEL-MAJOR ([g, d, l]) straight from the conv/silu
PSUM, and the host does the final [g, d, l] -> [B, L, HC, D] transpose during
unpack. This deletes the old pass3 (PE transpose-back) and its PSUM->SBUF
copies entirely. Squares for the RMSNorm stats are spread over DVE / ACT /
GpSimd (Pool) so no single engine eats the whole pass.

Per core, per 512-token chunk:
  1. stats: x^2 with 1/D folded accumulates to ms per token (engine per
     128-token block set by SQ_ENGINES); r = rsqrt(ms+eps) via bit-trick +
     1 Newton step on DVE (no ACT table swaps).
  2. pass1 (PE): Z[d, t] = X_blk^T @ diag(r) per 128x128 block -- transpose
     to channel-major with the RMSNorm scale folded in. DVE/ACT copy
     PSUM -> SBUF fp16 with a 6-column halo from the previous chunk.
  3. pass2 (PE): depthwise conv as 4 PSUM-accumulated matmuls
     diag(conv_w[k] * norm_w) @ Z[:, t - 6 + 2k].
  4. ACT Silu reads conv PSUM -> fp16 SBUF; DMA out channel-major.

I/O precision: host casts x to fp16 (halves input DMA); device returns fp16
y upcast to fp32 on host. End-to-end scale-relative error ~1e-3.
"""

import sys

if "/opt/trn_rl_repo" not in sys.path:
    sys.path.insert(0, "/opt/trn_rl_repo")

import numpy as np

B, L, HC, D = 4, 4096, 4, 1024
K, DIL = 4, 2
EPS = 1e-5
PAD = (K - 1) * DIL  # 6
NCORES = 8
NGROUPS = B * HC     # 16
GPC = NGROUPS // NCORES  # 2 groups per core

# tunables
TCH = 512            # tokens per chunk (= matmul moving free dim)
CPAIR = 2            # chunks whose stats are batched
SQ_ENGINES = ("vector", "act", "vector", "vector")  # per 128-token square blk
ZCOPY_ACT = 2        # of 8 dsubs per chunk, how many zcopies go to ACT

_prog_cache = {}


def build_program(L_=L, gpc=GPC, tch=TCH, cpair=CPAIR,
                  sq_engines=SQ_ENGINES, zcopy_act=ZCOPY_ACT):
    """Build the per-core Bacc program. Same program on all cores (SPMD)."""
    import concourse.bacc as bacc
    import concourse.tile as tile
    from concourse import mybir

    f32 = mybir.dt.float32
    f16 = mybir.dt.float16
    i32 = mybir.dt.int32
    AF = mybir.ActivationFunctionType
    ALU = mybir.AluOpType

    nblk = tch // 128
    dsub = D // 128
    nchunks = L_ // tch
    assert tch % 128 == 0 and L_ % tch == 0 and D % 128 == 0

    nc = bacc.Bacc()
    xin = nc.declare_dram_parameter("xin", [gpc, L_, D], f16, isOutput=False)
    wdg = nc.declare_dram_parameter("wdg", [gpc, K, dsub, 128, 128], f16,
                                    isOutput=False)
    idn = nc.declare_dram_parameter("idn", [128, 128], f16, isOutput=False)
    yout = nc.declare_dram_parameter("yout", [gpc, dsub, 128, L_], f16,
                                     isOutput=True)

    # views: token index t = c*tch + blk*128 + p
    xv = xin[:].rearrange("g (c blk p) d -> g c p blk d", blk=nblk, p=128)
    yv = yout[:].rearrange("g s p (c t) -> g c p s t", t=tch)
    wv = wdg[:].rearrange("g k s p m -> p g k s m")

    with tile.TileContext(nc) as tc:
        with (
            tc.tile_pool(name="pconst", bufs=1) as pconst,
            tc.tile_pool(name="px", bufs=8) as px,
            tc.tile_pool(name="pstat", bufs=3) as pstat,
            tc.tile_pool(name="pz", bufs=4) as pz,
            tc.tile_pool(name="py", bufs=3) as py,
            tc.tile_pool(name="pp1", bufs=2, space="PSUM") as pp1,
            tc.tile_pool(name="pp2", bufs=2, space="PSUM") as pp2,
        ):
            ident = pconst.tile([128, 128], f16)
            nc.sync.dma_start(out=ident[:], in_=idn[:])
            wsb = pconst.tile([128, gpc, K, dsub, 128], f16)
            nc.sync.dma_start(out=wsb[:], in_=wv)

            zt_prev = None
            for g in range(gpc):
                for c0 in range(0, nchunks, cpair):
                    cs = list(range(c0, min(c0 + cpair, nchunks)))
                    ncs = len(cs)
                    # ---- load chunks (natural [token, d] layout) ----
                    xhs = []
                    for c in cs:
                        xh = px.tile([128, nblk, D], f16, tag="xh")
                        nc.sync.dma_start(out=xh[:], in_=xv[g, c])
                        xhs.append(xh)

                    # ---- stats: ms = mean(x^2), batched per pair ----
                    ssq = pstat.tile([128, ncs, nblk], f32, tag="ssq")
                    for j in range(ncs):
                        for blk in range(nblk):
                            eng = sq_engines[blk % len(sq_engines)]
                            scr = pstat.tile([128, D], f16, tag="scr")
                            if eng == "act":
                                nc.scalar.activation(
                                    out=scr[:], in_=xhs[j][:, blk, :],
                                    func=AF.Square, scale=float(D) ** -0.5,
                                    accum_out=ssq[:, j, blk:blk + 1])
                            elif eng == "pool":
                                nc.gpsimd.scalar_tensor_tensor(
                                    out=scr[:], in0=xhs[j][:, blk, :],
                                    scalar=1.0 / D, in1=xhs[j][:, blk, :],
                                    op0=ALU.mult, op1=ALU.mult,
                                    accum_out=ssq[:, j, blk:blk + 1])
                            else:
                                nc.vector.scalar_tensor_tensor(
                                    out=scr[:], in0=xhs[j][:, blk, :],
                                    scalar=1.0 / D, in1=xhs[j][:, blk, :],
                                    op0=ALU.mult, op1=ALU.mult,
                                    accum_out=ssq[:, j, blk:blk + 1])
                    # r = rsqrt(ms+eps): bit trick + 1 Newton step on
                    # DVE (keeps Sqrt out of ACT -> zero table swaps)
                    v = pstat.tile([128, ncs, nblk], f32, tag="v")
                    nc.vector.tensor_scalar(
                        out=v[:], in0=ssq[:], scalar1=EPS, scalar2=None,
                        op0=ALU.add)
                    r = pstat.tile([128, ncs, nblk], f32, tag="r")
                    nc.vector.tensor_scalar(
                        out=r[:].bitcast(i32), in0=v[:].bitcast(i32),
                        scalar1=1, scalar2=None, op0=ALU.arith_shift_right)
                    nc.vector.tensor_scalar(
                        out=r[:].bitcast(i32), in0=r[:].bitcast(i32),
                        scalar1=-1, scalar2=0x5F3759DF,
                        op0=ALU.mult, op1=ALU.add)
                    for _ in range(1):
                        yy = pstat.tile([128, ncs, nblk], f32, tag="yy")
                        nc.vector.tensor_tensor(
                            out=yy[:], in0=r[:], in1=r[:], op=ALU.mult)
                        nc.vector.tensor_tensor(
                            out=yy[:], in0=yy[:], in1=v[:], op=ALU.mult)
                        nc.vector.tensor_scalar(
                            out=yy[:], in0=yy[:], scalar1=-0.5, scalar2=1.5,
                            op0=ALU.mult, op1=ALU.add)
                        rn = pstat.tile([128, ncs, nblk], f32, tag="rn")
                        nc.vector.tensor_tensor(
                            out=rn[:], in0=r[:], in1=yy[:], op=ALU.mult)
                        r = rn

                    # ---- pass1 per chunk: Z[d, t] = X^T diag(r) ----
                    zts = []
                    for j, c in enumerate(cs):
                        xh = xhs[j]
                        drt = pstat.tile([128, nblk, 128], f16, tag="drt")
                        for blk in range(nblk):
                            nc.vector.tensor_scalar_mul(
                                out=drt[:, blk, :], in0=ident[:],
                                scalar1=r[:, j, blk:blk + 1])

                        zt = pz.tile([128, dsub, PAD + tch], f16, tag="zt")
                        if c == 0:
                            nc.vector.memset(zt[:, :, 0:PAD], 0.0)
                        else:
                            nc.vector.tensor_copy(
                                out=zt[:, :, 0:PAD],
                                in_=zt_prev[:, :, tch:tch + PAD])
                        for s in range(dsub):
                            zp = pp1.tile([128, tch], f32, tag="zp")
                            for blk in range(nblk):
                                nc.tensor.matmul(
                                    zp[:, blk * 128:(blk + 1) * 128],
                                    lhsT=xh[:, blk, s * 128:(s + 1) * 128],
                                    rhs=drt[:, blk, :],
                                    start=True, stop=True)
                            if s < zcopy_act:
                                nc.scalar.copy(
                                    out=zt[:, s, PAD:PAD + tch], in_=zp[:])
                            else:
                                nc.vector.tensor_copy(
                                    out=zt[:, s, PAD:PAD + tch], in_=zp[:])
                        zt_prev = zt
                        zts.append(zt)

                    # ---- pass2 paired: conv matmuls share ldweights;
                    #      silu writes fp16 channel-major, DMA straight out
                    yhs = [py.tile([128, dsub, tch], f16, tag=f"yh{j}",
                                   name=f"yh{j}_{g}_{c0}")
                           for j in range(ncs)]
                    for s in range(dsub):
                        yps = [pp2.tile([128, tch], f32, tag=f"yp{j}",
                                        name=f"yp{j}_{g}_{c0}_{s}")
                               for j in range(ncs)]
                        for k in range(K):
                            for j in range(ncs):
                                nc.tensor.matmul(
                                    yps[j][:],
                                    lhsT=wsb[:, g, k, s, :],
                                    rhs=zts[j][:, s, k * DIL:k * DIL + tch],
                                    start=(k == 0), stop=(k == K - 1))
                        for j in range(ncs):
                            nc.scalar.activation(out=yhs[j][:, s, :],
                                                 in_=yps[j][:], func=AF.Silu)

                    for j, c in enumerate(cs):
                        nc.sync.dma_start(out=yv[g, c], in_=yhs[j][:])
    nc.compile()
    return nc


def _host_pack(x, norm_weight, conv_weight):
    """Shard inputs across cores; fold norm weight into conv diagonals."""
    dsub = D // 128
    xg = np.ascontiguousarray(x.transpose(0, 2, 1, 3)).reshape(NGROUPS, L, D)
    xg = xg.astype(np.float16)
    conv_w = conv_weight.reshape(HC, D, K)            # [hc, d, k]
    weff = conv_w * norm_weight[:, :, None]           # [hc, d, k]
    wr = weff.transpose(0, 2, 1).reshape(HC, K, dsub, 128)  # [hc, k, s, i]
    eye = np.eye(128, dtype=np.float32)
    wdiag = (wr[..., None] * eye).astype(np.float16)  # [hc, K, s, 128, 128]
    idn = eye.astype(np.float16)

    in_maps = []
    for i in range(NCORES):
        gs = [i * GPC + j for j in range(GPC)]
        in_maps.append({
            "xin": np.ascontiguousarray(xg[gs[0]:gs[-1] + 1]),
            "wdg": np.ascontiguousarray(
                np.stack([wdiag[g % HC] for g in gs])),
            "idn": idn,
        })
    return in_maps


def _host_unpack(results):
    # yout per core: [gpc, dsub, 128, L] channel-major
    ys = np.concatenate([r["yout"] for r in results], axis=0)  # [16, 8, 128, L]
    y = ys.reshape(B, HC, D, L).transpose(0, 3, 1, 2)          # [B, L, HC, D]
    return np.ascontiguousarray(y.astype(np.float32))


def _get_prog():
    key = (L, GPC, TCH, CPAIR, SQ_ENGINES, ZCOPY_ACT)
    if key not in _prog_cache:
        _prog_cache[key] = build_program()
    return _prog_cache[key]


def kernel(x, norm_weight, conv_weight, _trace=False, _trace_kwargs=None):
    from concourse.bass_utils import run_bass_kernel_spmd

    x = np.asarray(x, dtype=np.float32)
    norm_weight = np.asarray(norm_weight, dtype=np.float32)
    conv_weight = np.asarray(conv_weight, dtype=np.float32)

    nc = _get_prog()
    in_maps = _host_pack(x, norm_weight, conv_weight)
    res = run_bass_kernel_spmd(
        nc, in_maps, list(range(NCORES)),
        trace=_trace, **(_trace_kwargs or {}))
    out = _host_unpack(res.results)
    if _trace:
        return out, res
    return out
